# revision 1
# baseline (speedup 1.0000x reference)
"""MultiHeadHierarchicalAttentionBlock Trainium2 kernel (8 cores).

Sharding: core = (b, j): batch b in {0,1}, sequence block j in {0..3} of
T=1024 tokens.  On-chip layout is transposed: [channels (partition),
tokens (free)].

Per head i (sequentially dependent), software-pipelined across heads:
  xiT = x_slice + prevT (+ones row)
  QlT/KlT directly from xiT via host-folded (qlw@qw) weights (Q/K never
  materialized); V in [token, ch] layout; all matmuls f32r or bf16.
  AllGather (KlT, V'-with-ones) across the 4 cores of this batch;
  remote slots located via host-supplied rslots registers, own chunks
  processed from local SBUF while the collective flies.
  64 (chunk, q-half) units: scoresT [128,512] (bf16 matmul) -> exp (ACT,
  groups of 3/2 units) -> V'^T @ exp accumulating [65,512] per q-half
  (ones column of V' accumulates the softmax denominator in row 64).
  Per-q-half tail: recip -> gpsimd partition-broadcast -> normalize ->
  out-proj -> next head's xiT/Ql/Kl/V -- hidden under remaining units.
BatchNorm sync: per-head AllReduce of bn_stats (hidden under next head),
rstd via DVE-only Newton rsqrt (no ACT table switches).  FFN with exact
GELU; BN2 sync; residuals.
"""
import sys
sys.path.insert(0, '/opt/trn_rl_repo')
import numpy as np

import concourse.bass as bass
from concourse import bacc, tile, mybir

F32 = mybir.dt.float32
F32R = mybir.dt.float32r
BF16 = mybir.dt.bfloat16
I32 = mybir.dt.int32
AF = mybir.ActivationFunctionType
ALU = mybir.AluOpType

N_CORES = 8
B, C, H, W = 2, 256, 64, 64
S = H * W
T = S // 4           # 1024 tokens per core
BN_EPS = 1e-5
GROUPS = [[0, 1, 2, 3], [4, 5, 6, 7]]
RSQRT_MAGIC = 0x5F3759DF


# ---------------------------------------------------------------- host side
def prep_host_inputs(x, qw, qb, kw, kb, vw, vb, qlw, klw, ow, ob,
                     f1w, f1b, f2w, f2b, g1, b1, g2, b2):
    x = np.asarray(x, np.float32)
    # Q/K are pure intermediates: fold  Ql = (qlw @ qw) @ xi + (qlw @ qb)
    whead = np.zeros((65, 4, 96), np.float32)
    for i in range(4):
        qwc = (np.asarray(qlw)[i].astype(np.float64)
               @ np.asarray(qw)[i].astype(np.float64))
        kwc = (np.asarray(klw)[i].astype(np.float64)
               @ np.asarray(kw)[i].astype(np.float64))
        whead[0:64, i, 0:16] = qwc.T
        whead[64, i, 0:16] = qwc @ np.asarray(qb)[i].astype(np.float64)
        whead[0:64, i, 16:32] = kwc.T
        whead[64, i, 16:32] = kwc @ np.asarray(kb)[i].astype(np.float64)
        whead[0:64, i, 32:96] = np.asarray(vw)[i].T
        whead[64, i, 32:96] = np.asarray(vb)[i]
    wo_p = np.zeros((65, 4, 64), np.float32)
    ob_col = np.zeros((128, 4), np.float32)
    for i in range(4):
        wo_p[0:64, i, :] = np.asarray(ow)[i].T
        wo_p[64, i, :] = np.asarray(ob)[i]
        ob_col[0:64, i] = np.asarray(ob)[i]
        ob_col[64:128, i] = np.asarray(ob)[i]
    wf1 = np.ascontiguousarray(np.asarray(f1w).T)                  # [256,1024]
    bf1 = np.ascontiguousarray(np.asarray(f1b).reshape(8, 128).T)  # [128,8]
    wf2 = np.ascontiguousarray(np.asarray(f2w).T)                  # [1024,256]
    # f2b dropped: a per-channel bias before BN2 cancels exactly.
    gb1 = np.stack([np.asarray(g1).reshape(2, 128).T,
                    np.asarray(b1).reshape(2, 128).T], axis=-1)
    gb2 = np.stack([np.asarray(g2).reshape(2, 128).T,
                    np.asarray(b2).reshape(2, 128).T], axis=-1)
    shared = {
        "whead": whead, "wo_p": wo_p, "ob_col": ob_col,
        "wf1": wf1, "bf1": bf1, "wf2": wf2,
        "gb1": np.ascontiguousarray(gb1, dtype=np.float32),
        "gb2": np.ascontiguousarray(gb2, dtype=np.float32),
    }
    xr = x.reshape(B, C, S)
    in_maps = []
    for core in range(N_CORES):
        b, j = divmod(core, 4)
        m = dict(shared)
        m["xb"] = np.ascontiguousarray(xr[b, :, j * T:(j + 1) * T])
        m["rslots"] = np.array([[(j + 1) % 4, (j + 2) % 4, (j + 3) % 4]],
                               np.uint32)
        in_maps.append(m)
    return in_maps


def assemble_output(results):
    out = np.zeros((B, C, S), np.float32)
    for core in range(N_CORES):
        b, j = divmod(core, 4)
        out[b, :, j * T:(j + 1) * T] = results[core]["out"]
    return out.reshape(B, C, H, W)


# ---------------------------------------------------------------- helpers
def _memset_r(nc, ap, val):
    nc.vector.memset(ap.bitcast(F32), val)


def _rsqrt_dve(nc, G, y_out, x_in, tag):
    """y_out = 1/sqrt(x_in) on DVE only ([r, f] APs, aligned partitions)."""
    r = x_in.shape[0]
    f = x_in.shape[1] if len(x_in.shape) > 1 else 1
    b = x_in.base_partition()
    work = G["work"]
    iv = work.tile([128, f], I32, tag=tag + "i")
    nc.vector.tensor_scalar(out=iv[b:b + r, :], in0=x_in.bitcast(I32),
                            scalar1=1, scalar2=None,
                            op0=ALU.logical_shift_right)
    nc.vector.tensor_tensor(out=y_out.bitcast(I32),
                            in0=G["magic"][b:b + r, 0:f],
                            in1=iv[b:b + r, :], op=ALU.subtract)
    tmp = work.tile([128, f], F32, tag=tag + "t")
    tm = tmp[b:b + r, :]
    for _ in range(2):
        nc.vector.tensor_mul(tm, y_out, y_out)
        nc.vector.scalar_tensor_tensor(out=tm, in0=tm, scalar=-0.5, in1=x_in,
                                       op0=ALU.mult, op1=ALU.mult)
        nc.vector.tensor_scalar_add(tm, tm, 1.5)
        nc.vector.tensor_mul(y_out, y_out, tm)


def _collective_ar(nc, din, dout, fake, eng=None):
    if fake:
        (eng or nc.sync).dma_start(out=dout[:], in_=din[:])
    else:
        nc.gpsimd.collective_compute(
            "AllReduce", ALU.add, replica_groups=[list(range(N_CORES))],
            ins=[din[:].opt()], outs=[dout[:].opt()])


def _kl_half(nc, G, i, h, xiT, qlkl):
    psum, whead = G["psum"], G["whead"]
    sl = slice(512 * h, 512 * h + 512)
    kl_ps = psum.tile([16, 512], F32, tag="stg")
    nc.tensor.matmul(kl_ps[:], whead[:, i, 16:32], xiT[:, sl],
                     start=True, stop=True)
    nc.vector.tensor_copy(qlkl[:, 1, sl], kl_ps[:])


def _ql_half(nc, G, i, h, xiT, qlkl):
    psum, whead = G["psum"], G["whead"]
    sl = slice(512 * h, 512 * h + 512)
    ql_ps = psum.tile([16, 512], F32, tag="stg")
    nc.tensor.matmul(ql_ps[:], whead[:, i, 0:16], xiT[:, sl],
                     start=True, stop=True)
    nc.vector.tensor_copy(qlkl[:, 0, sl], ql_ps[:])


def _proj_half(nc, G, i, h, xiT, qlkl):
    _kl_half(nc, G, i, h, xiT, qlkl)
    _ql_half(nc, G, i, h, xiT, qlkl)


def _v_half(nc, G, i, h, xiT, v_own):
    """V chunks 4h..4h+3 of head i into v_own (bf16, with ones cols)."""
    psum, whead = G["psum"], G["whead"]
    v_ps = psum.tile([128, 256], F32, tag="stg")
    for c in range(4 * h, 4 * h + 4):
        nc.tensor.matmul(v_ps[:, 64 * (c - 4 * h):64 * (c - 4 * h) + 64],
                         xiT[:, 128 * c:128 * c + 128], whead[:, i, 32:96],
                         start=True, stop=True)
    nc.vector.tensor_copy(
        v_own[:, 4 * h:4 * h + 4, 0:64],
        v_ps[:].rearrange("p (c f) -> p c f", f=64))
    nc.vector.memset(v_own[:, 4 * h:4 * h + 4, 64:65], 1.0)


AG_HN = 41472            # per-half AG payload: 16*512 Kl + 128*260 V'


def _launch_ag_half(nc, G, h, qlkl, v_own):
    """Ship token-half h's Kl + V' chunks to the group; load 3 remote slots.

    Fake path: 3 same-size copies (one per remote rank = ring-AG wire
    traffic) split across the SP and Pool DMA queues."""
    work, dram, slots, slots_p, fake = (G["work"], G["dram"], G["slots"],
                                        G["slots_p"], G["fake"])
    ag_in = dram.tile([AG_HN], BF16, tag=f"agin{h}")
    ag_out = dram.tile([4, AG_HN], BF16, tag=f"agout{h}")
    nc.sync.dma_start(out=ag_in[0:8192].rearrange("(p t) -> p t", p=16),
                      in_=qlkl[:, 1, 512 * h:512 * h + 512])
    nc.gpsimd.dma_start(
        out=ag_in[8192:AG_HN].rearrange("(p c) -> p c", p=128),
        in_=v_own[:, 4 * h:4 * h + 4, :].rearrange("p c f -> p (c f)"))
    if fake:
        for r in range(3):
            eng = nc.sync if r % 2 == 0 else nc.gpsimd
            eng.dma_start(out=ag_out[r + 1].rearrange("(p c) -> p c", p=128),
                          in_=ag_in[:].rearrange("(p c) -> p c", p=128))
    else:
        nc.gpsimd.collective_compute(
            "AllGather", ALU.bypass, replica_groups=GROUPS,
            ins=[ag_in[:].opt()], outs=[ag_out[:].opt()])
    klf = work.tile([16, 3, 512], BF16, tag=f"klf{h}")
    vt = work.tile([128, 3, 260], BF16, tag=f"vt{h}")
    for k in range(3):
        reg = bass.ds(slots[k], 1)
        nc.sync.dma_start(
            out=klf[:, k, :],
            in_=ag_out[reg, 0:8192].rearrange("a (p t) -> (a p) t", p=16))
        if k % 2 == 0:
            regp = bass.ds(slots_p[k], 1)
            nc.gpsimd.dma_start(
                out=vt[:, k, :],
                in_=ag_out[regp, 8192:AG_HN].rearrange(
                    "a (p c) -> (a p) c", p=128))
        else:
            nc.sync.dma_start(
                out=vt[:, k, :],
                in_=ag_out[reg, 8192:AG_HN].rearrange(
                    "a (p c) -> (a p) c", p=128))
    return klf, vt


def _bn1_stats_half(nc, G, i, a):
    """BN1 stats for token-half a of head i's cc (emit as soon as that
    half's cc is written, so head 3's exposed chain shortens)."""
    hi, lo = divmod(i, 2)
    p0, p1 = 64 * lo, 64 * lo + 64
    work = G["work"]
    if a == 0:
        bstat = work.tile([128, 2, 6], F32, tag="bstat")
        G["bstat_h"] = bstat
    else:
        bstat = G["bstat_h"]
    ccs3 = G["cc"][p0:p1, hi, :].rearrange("p (a t) -> p a t", a=2)
    nc.vector.bn_stats(bstat[p0:p1, a, :], ccs3[:, a, :])
    return bstat


def _bn1_head_stats(nc, G, i, bstat):
    """Per-head BN1 stats + AllReduce launch.  Returns a closure with the
    post-AR math, to be emitted once the AR has surely landed."""
    hi, lo = divmod(i, 2)
    p0, p1 = 64 * lo, 64 * lo + 64
    work, dram, stat = G["work"], G["dram"], G["stat"]
    cc, x_sb, gb1, out1 = G["cc"], G["x_sb"], G["gb1"], G["out1"]
    ccs = cc[p0:p1, hi, :]
    mv = work.tile([128, 2], F32, tag="mv")
    nc.vector.bn_aggr(mv[p0:p1], bstat[p0:p1])
    m2a = work.tile([128, 1], F32, tag="m2")
    sl = stat[p0:p1, hi, :]
    nc.vector.tensor_scalar_mul(sl[:, 0:1], mv[p0:p1, 0:1], 0.125)
    nc.vector.scalar_tensor_tensor(
        out=m2a[p0:p1], in0=mv[p0:p1, 0:1], scalar=1.0, in1=mv[p0:p1, 0:1],
        op0=ALU.mult, op1=ALU.mult)
    nc.vector.tensor_scalar(
        out=sl[:, 1:2], in0=mv[p0:p1, 1:2], scalar1=m2a[p0:p1], scalar2=0.125,
        op0=ALU.add, op1=ALU.mult)
    ar_in = dram.tile([64, 2], F32, tag="arin")
    ar_out = dram.tile([64, 2], F32, tag="arout")
    nc.sync.dma_start(out=ar_in[:, :], in_=sl[:, :])
    _collective_ar(nc, ar_in, ar_out, G["fake"])

    def post():
        g = work.tile([128, 2], F32, tag="gstat")
        nc.sync.dma_start(out=g[p0:p1, :], in_=ar_out[:, :])
        scl = work.tile([128, 2], F32, tag="scl")  # [:,0]=rstd*g1 [:,1]=mean
        m2 = work.tile([128, 1], F32, tag="m2")
        nc.vector.scalar_tensor_tensor(
            out=m2[p0:p1], in0=g[p0:p1, 0:1], scalar=1.0, in1=g[p0:p1, 0:1],
            op0=ALU.mult, op1=ALU.mult)
        vtmp = work.tile([128, 1], F32, tag="vtmp")
        nc.vector.tensor_scalar(
            out=vtmp[p0:p1], in0=g[p0:p1, 1:2], scalar1=m2[p0:p1],
            scalar2=BN_EPS, op0=ALU.subtract, op1=ALU.add)
        nc.vector.tensor_scalar_mul(scl[p0:p1, 1:2], g[p0:p1, 0:1], 1.0)
        rstd = work.tile([128, 1], F32, tag="rstd")
        _rsqrt_dve(nc, G, rstd[p0:p1], vtmp[p0:p1], "rq")
        nc.vector.scalar_tensor_tensor(
            out=scl[p0:p1, 0:1], in0=rstd[p0:p1], scalar=1.0,
            in1=gb1[p0:p1, hi, 0:1], op0=ALU.mult, op1=ALU.mult)
        t = work.tile([128, T], F32, tag="o1t")
        if i == 3:
            # last head's chain is exposed: ACT (idle here) does the affine
            mb = work.tile([128, 1], F32, tag="mb1")
            nc.vector.scalar_tensor_tensor(
                out=mb[p0:p1], in0=scl[p0:p1, 1:2], scalar=-1.0,
                in1=scl[p0:p1, 0:1], op0=ALU.mult, op1=ALU.mult)
            nc.vector.tensor_tensor(out=mb[p0:p1], in0=mb[p0:p1],
                                    in1=gb1[p0:p1, hi, 1:2], op=ALU.add)
            nc.scalar.activation(t[p0:p1, :], ccs, AF.Identity,
                                 bias=mb[p0:p1], scale=scl[p0:p1, 0:1])
            # conv1's bf16 input first (it gates the FFN), f32 after
            nc.vector.tensor_tensor(
                out=G["out1b"][p0:p1, hi, :], in0=t[p0:p1, :],
                in1=x_sb[p0:p1, hi, :], op=ALU.add)
            nc.vector.tensor_tensor(
                out=out1[p0:p1, hi, :], in0=t[p0:p1, :],
                in1=x_sb[p0:p1, hi, :], op=ALU.add)
        else:
            nc.vector.tensor_scalar(
                out=t[p0:p1, :], in0=ccs, scalar1=scl[p0:p1, 1:2],
                scalar2=scl[p0:p1, 0:1], op0=ALU.subtract, op1=ALU.mult)
            nc.vector.scalar_tensor_tensor(
                out=out1[p0:p1, hi, :], in0=t[p0:p1, :],
                scalar=gb1[p0:p1, hi, 1:2], in1=x_sb[p0:p1, hi, :],
                op0=ALU.add, op1=ALU.add)
            # bf16 shadow of out1 for the bf16 FFN conv1
            nc.vector.tensor_copy(G["out1b"][p0:p1, hi, :],
                                  out1[p0:p1, hi, :])
    return post


def _head(nc, G, i, hstate):
    """Head i's unit loop interleaved with its tail + head i+1's prep."""
    hi, lo = divmod(i, 2)
    (xiT, qlkl, v_own, klfA, vtA, klfB, vtB, out_pair, started, n_left, gi,
     skip0, pending_post) = hstate
    work, wk1, expp, psum = G["work"], G["wk1"], G["expp"], G["psum"]
    x_sb, whead, wo, cc = G["x_sb"], G["whead"], G["wo"], G["cc"]

    # query-half-serialized (heads 1-3): all qh0 units, half-0 tail, then
    # qh1.  The inactive attnV accumulator bank doubles as a third score
    # buffer so the next exp's scores are always prefetched.  Head 0
    # instead fronts all 16 own units ((3,2) grouping, no third buffer)
    # to maximize runway for its fully-exposed first AllGather.
    first = (i == 0)
    units = []
    if first:
        for qh in range(2):
            for ch in range(8):
                units.append(("own", ch, qh))
        for qh in range(2):
            for half in range(2):
                for k in range(3):
                    for ch in range(4 * half, 4 * half + 4):
                        units.append((k, ch, qh))
    else:
        for qh in range(2):
            for ch in range(8):
                units.append(("own", ch, qh))
            # remote: A-half chunks (0-3) before B-half (4-7), so the B
            # half-AllGather has 12 more units of runway to land.
            for half in range(2):
                for k in range(3):
                    for ch in range(4 * half, 4 * half + 4):
                        units.append((k, ch, qh))

    st = {"gi": gi, "started": started, "n_left": n_left}

    def emit_units(ulist, st, qlkl_, kls_, v_own_, vts_, out_pair_, ctag):
        # Build 3/2/1-unit groups cycling over the scA/scB/<inactive-out>
        # PSUM buffers (3/2 only when ctag is None), emitted with the score
        # matmuls one group ahead of the exp+attnV, so the next exp's input
        # is already in flight on PE when the previous exp retires (PE is
        # in-order).
        groups = []
        g0 = 0
        while g0 < len(ulist):
            if ctag is None:
                ph = st["gi"] % 2
                cap = (3, 2)[ph]
                tags = (("scA", "exA"), ("scB", "exB"))[ph]
            else:
                ph = st["gi"] % 3
                cap = (3, 2, 1)[ph]
                tags = (("scA", "exA"), ("scB", "exB"),
                        (ctag, "exC"))[ph]
            grp = ulist[g0:g0 + cap]
            g0 += cap
            groups.append((grp, tags[0], tags[1]))
            st["gi"] += 1

        def score_mm(grp, gtag):
            sc_ps = psum.tile([128, 512 * len(grp)], F32, tag=gtag)
            for ui, (src_k, ch, qh) in enumerate(grp):
                if src_k == "own":
                    kl = qlkl_[:, 1, 128 * ch:128 * ch + 128]
                else:
                    c = ch % 4
                    kl = kls_[ch // 4][:, src_k, 128 * c:128 * c + 128]
                nc.tensor.matmul(sc_ps[:, 512 * ui:512 * ui + 512], kl,
                                 qlkl_[:, 0, 512 * qh:512 * qh + 512],
                                 start=True, stop=True)
            return sc_ps

        def exp_av(grp, extag, sc_ps):
            ex = expp.tile([128, 512 * len(grp)], BF16, tag=extag)
            nc.scalar.activation(ex[:], sc_ps[:], AF.Exp, scale=0.25)
            for ui, (src_k, ch, qh) in enumerate(grp):
                if src_k == "own":
                    vt = v_own_[:, ch, :]
                else:
                    c = ch % 4
                    vt = vts_[ch // 4][:, src_k, 65 * c:65 * c + 65]
                st["n_left"][qh] -= 1
                nc.tensor.matmul(
                    out_pair_[qh][:], vt, ex[:, 512 * ui:512 * ui + 512],
                    start=not st["started"][qh],
                    stop=(st["n_left"][qh] == 0),
                    skip_group_check=True)
                st["started"][qh] = True

        pend = None
        for grp, gtag, extag in groups:
            sc = score_mm(grp, gtag)
            if pend is not None:
                exp_av(*pend)
            pend = (grp, extag, sc)
        if pend is not None:
            exp_av(*pend)

    def tail_norm_half(h, recip, bcast, o_sb):
        sl = slice(512 * h, 512 * h + 512)
        # copy UNNORMALIZED out for the projection; recip/bcast concurrent
        nc.vector.tensor_copy(o_sb[0:64, sl], out_pair[h][0:64, :])
        nc.vector.reciprocal(recip[0:1, sl], out_pair[h][64:65, :])
        nc.gpsimd.partition_broadcast(bcast[:, sl], recip[0:1, sl])

    def tail_proj_half(h, o_sb, xiT_n, qlkl_n, v_own_n, defer_ql=False):
        # prev_norm = (wo @ out_un) * bcast  (per-q scaling commutes)
        sl = slice(512 * h, 512 * h + 512)
        nlo = 1 - lo
        nhi = (i + 1) // 2
        prev_ps = psum.tile([64, 512], F32, tag="stg")
        nc.tensor.matmul(prev_ps[:], wo[0:64, i, :], o_sb[0:64, sl],
                         start=True, stop=True)
        ccs = cc[64 * lo:64 * lo + 64, hi, sl]
        t128 = G["t128"]
        # t (at next head's partition base) = prev_un * recip_bcast
        nc.vector.tensor_mul(t128[64 * nlo:64 * nlo + 64, sl], prev_ps[:],
                             bcast[:, sl])
        if xiT_n is not None:
            nc.vector.tensor_add(
                xiT_n[0:64, sl], t128[64 * nlo:64 * nlo + 64, sl],
                G["xob"][64 * nlo:64 * nlo + 64, nhi, sl])
            _kl_half(nc, G, i + 1, h, xiT_n, qlkl_n)
            _v_half(nc, G, i + 1, h, xiT_n, v_own_n)
            if not defer_ql:
                _ql_half(nc, G, i + 1, h, xiT_n, qlkl_n)
        # cc = prev_norm + ob  (off critical path)
        nc.vector.tensor_scalar(
            out=ccs, in0=t128[64 * nlo:64 * nlo + 64, sl],
            scalar1=G["obc"][64 * nlo:64 * nlo + 64, i:i + 1], scalar2=None,
            op0=ALU.add)

    kls, vts = (klfA, klfB), (vtA, vtB)
    # ---- qh0 units (first `skip0` pre-emitted by prev head); the idle
    # outB bank serves as the third score buffer during this phase.
    qh0_end = 40 if first else 32
    mid = 28 if first else 20
    ctag0 = None if first else "outB"
    emit_units(units[skip0:mid], st, qlkl, kls, v_own, vts, out_pair, ctag0)
    if pending_post is not None:
        pending_post()          # previous head's BN1 post-AR math
    emit_units(units[mid:qh0_end], st, qlkl, kls, v_own, vts, out_pair,
               ctag0)

    # ---- half-0 tail + next-head half-0 prep (hidden under qh1 units)
    recip = wk1.tile([1, 1024], F32, tag="recip")
    bcast = wk1.tile([64, 1024], F32, tag="bcast")
    o_sb = wk1.tile([65, 1024], F32R, tag="osb")
    t128 = work.tile([128, 1024], F32, tag="t128")
    G["t128"] = t128
    if i < 3:
        xiT_n = work.tile([65, T], F32R, tag="xiT")
        if i == 0:
            _memset_r(nc, xiT_n[64:65, :], 1.0)
        qlkl_n = work.tile([16, 2, T], BF16, tag="qlkl")
        v_own_n = work.tile([128, 8, 65], BF16, tag="vown")
    else:
        xiT_n = qlkl_n = v_own_n = None
    tail_norm_half(0, recip, bcast, o_sb)
    if not first:
        # qh1's accumulator: allocated only now, after qh0's score C-tiles
        # are done with the outB bank
        outb_own = psum.tile([65, 512], F32, tag="outB")
        out_pair = (out_pair[0], outb_own)
    tail_proj_half(0, o_sb, xiT_n, qlkl_n, v_own_n)
    bstat = _bn1_stats_half(nc, G, i, 0)
    if i < 3:
        # next head's Kl/V token-half 0 just computed: ship it now
        klfA_n, vtA_n = _launch_ag_half(nc, G, 0, qlkl_n, v_own_n)

    # ---- qh1 units (outA bank is now the third score buffer)
    emit_units(units[qh0_end:64], st, qlkl, kls, v_own, vts, out_pair,
               None if first else "outA")

    # ---- half-1 normalization (frees outB early)
    tail_norm_half(1, recip, bcast, o_sb)

    # ---- next head's first units run while the tail chain drains: 4 own
    # before the half-1 tail, then the 12 remote-A units (their AG_A data
    # landed mid-head) after AG_B is on its way.
    nxt_state = None
    if i < 3:
        outa = psum.tile([65, 512], F32, tag="outA")
        st_n = {"gi": st["gi"], "started": [False, False],
                "n_left": {0: 32, 1: 32}}
        emit_units([("own", ch, 0) for ch in range(4)], st_n,
                   qlkl_n, None, v_own_n, None, (outa, None), "outB")
    tail_proj_half(1, o_sb, xiT_n, qlkl_n, v_own_n, defer_ql=True)
    if i < 3:
        klfB_n, vtB_n = _launch_ag_half(nc, G, 1, qlkl_n, v_own_n)
        _ql_half(nc, G, i + 1, 1, xiT_n, qlkl_n)
        nxt_state = (xiT_n, qlkl_n, v_own_n, klfA_n, vtA_n, klfB_n, vtB_n,
                     (outa, None), st_n["started"], st_n["n_left"],
                     st_n["gi"], 4)

    _bn1_stats_half(nc, G, i, 1)
    post = _bn1_head_stats(nc, G, i, bstat)
    if nxt_state is not None:
        nxt_state = nxt_state + (post,)
    else:
        post()                  # last head: emit immediately
    return nxt_state


def _ffn_tail(nc, G, out_e):
    """FFN + BN2 + final residual.

    conv1 (bf16) -> GELU stream with conv2 (bf16) pipelined one hidden-chunk
    behind for 3 of the 4 (out-chunk, token-half) quarters; the 4th runs
    right after the stream.  BN2 stats AR per out-chunk, fused affine +
    residual on DVE, per-chunk output DMA."""
    work, wk1, state, psum, dram = (G["work"], G["wk1"], G["state"],
                                    G["psum"], G["dram"])
    wf1b, bf1, wf2b, gb2 = G["wf1b"], G["bf1"], G["wf2b"], G["gb2"]
    out1, out1b = G["out1"], G["out1b"]

    # dummy gelu: pulls the exp->gelu ACT table switch into head-3's drain
    tl = work.tile([128, 1], F32, tag="tl")
    nc.scalar.activation(tl[:], bf1[:, 0:1], AF.Gelu)

    h_all = state.tile([128, 8, T], BF16, tag="hall")
    h2 = state.tile([128, 2, T], F32, tag="h2")
    # conv2 quarter accumulators: (m, th) -> psum region
    oq00 = psum.tile([128, 512], F32, tag="outA")
    oq10 = psum.tile([128, 512], F32, tag="outB")
    oq01 = psum.tile([128, 512], F32, tag="stg")
    oq = {(0, 0): oq00, (1, 0): oq10, (0, 1): oq01}

    def conv2_step(k):
        for (m, th), ps in oq.items():
            nc.tensor.matmul(ps[:], wf2b[:, k, 128 * m:128 * m + 128],
                             h_all[:, k, 512 * th:512 * th + 512],
                             start=(k == 0), stop=(k == 7))

    for k in range(8):
        h_ps = psum.tile([128, 1024], F32, tag="scA" if k % 2 == 0 else "scB")
        for c in range(2):
            for hh in range(2):
                nc.tensor.matmul(
                    h_ps[:, 512 * hh:512 * hh + 512],
                    wf1b[:, c, 128 * k:128 * k + 128],
                    out1b[:, c, 512 * hh:512 * hh + 512],
                    start=(c == 0), stop=(c == 1))
        nc.scalar.activation(h_all[:, k, :], h_ps[:], AF.Gelu,
                             bias=bf1[:, k:k + 1], scale=1.0)
        if k > 0:
            conv2_step(k - 1)    # one chunk behind: GELU k-1 already landed
    conv2_step(7)

    # 4th quarter (m=1, th=1) + per-m stats/AR as each m completes
    stat2 = work.tile([128, 2, 2], F32, tag="stat2")
    ars = []
    o11 = None
    for m in range(2):
        if m == 0:
            # ACT does the PSUM->SBUF copies (free after last GELU)
            nc.scalar.activation(h2[:, 0, 0:512], oq[(0, 0)][:], AF.Identity)
            nc.scalar.activation(h2[:, 0, 512:1024], oq[(0, 1)][:],
                                 AF.Identity)
            # m=1 th=1 accumulates in the scA bank (free after GELU 6)
            o11 = psum.tile([128, 512], F32, tag="scA")
            for k in range(8):
                nc.tensor.matmul(o11[:], wf2b[:, k, 128:256],
                                 h_all[:, k, 512:1024],
                                 start=(k == 0), stop=(k == 7))
        else:
            nc.scalar.activation(h2[:, 1, 0:512], oq[(1, 0)][:], AF.Identity)
            nc.scalar.activation(h2[:, 1, 512:1024], o11[:], AF.Identity)
        bstat = work.tile([128, 2, 6], F32, tag="bstat")
        h23 = h2[:, m, :].rearrange("p (a t) -> p a t", a=2)
        for a in range(2):
            nc.vector.bn_stats(bstat[:, a, :], h23[:, a, :])
        mv = work.tile([128, 2], F32, tag="mv")
        nc.vector.bn_aggr(mv[:], bstat[:])
        m2 = work.tile([128, 1], F32, tag="m2")
        sl = stat2[:, m, :]
        nc.vector.tensor_scalar_mul(sl[:, 0:1], mv[:, 0:1], 0.125)
        nc.vector.scalar_tensor_tensor(
            out=m2[:], in0=mv[:, 0:1], scalar=1.0, in1=mv[:, 0:1],
            op0=ALU.mult, op1=ALU.mult)
        nc.vector.tensor_scalar(
            out=sl[:, 1:2], in0=mv[:, 1:2], scalar1=m2[:], scalar2=0.125,
            op0=ALU.add, op1=ALU.mult)
        # AllReduce this tile's stats immediately, m0 and m1 on separate
        # DMA queues
        eng = nc.sync if m == 0 else nc.gpsimd
        arm_in = dram.tile([128, 2], F32, tag=f"ar2in{m}")
        arm_out = dram.tile([128, 2], F32, tag=f"ar2out{m}")
        eng.dma_start(out=arm_in[:, :], in_=sl[:, :])
        _collective_ar(nc, arm_in, arm_out, G["fake"], eng=eng)
        ars.append(arm_out)

    # both BN2 rsqrt chains batched 2-wide
    gg = work.tile([128, 2, 2], F32, tag="gstat2")
    nc.sync.dma_start(out=gg[:, 0, :], in_=ars[0][:, :])
    nc.gpsimd.dma_start(out=gg[:, 1, :], in_=ars[1][:, :])
    m2 = work.tile([128, 2], F32, tag="m2b")
    nc.vector.tensor_mul(m2[:], gg[:, :, 0], gg[:, :, 0])
    vtmp = work.tile([128, 2], F32, tag="vtmpb")
    nc.vector.tensor_sub(vtmp[:], gg[:, :, 1], m2[:])
    nc.vector.tensor_scalar_add(vtmp[:], vtmp[:], BN_EPS)
    rstd = work.tile([128, 2], F32, tag="rstdb")
    _rsqrt_dve(nc, G, rstd[:], vtmp[:], "rq2")
    scl = work.tile([128, 2], F32, tag="scl2")
    nc.vector.tensor_mul(scl[:], rstd[:], gb2[:, :, 0])
    # mb = b2 - mean*scl; fin = (h2*scl + mb) + out1
    mb = work.tile([128, 2], F32, tag="mb2")
    nc.vector.scalar_tensor_tensor(
        out=mb[:], in0=gg[:, :, 0], scalar=-1.0, in1=scl[:],
        op0=ALU.mult, op1=ALU.mult)
    nc.vector.tensor_tensor(out=mb[:], in0=mb[:], in1=gb2[:, :, 1],
                            op=ALU.add)
    outv = out_e.ap().rearrange("(k p) t -> p k t", p=128)
    for k in range(2):
        tmp = wk1.tile([128, T], F32, tag=f"tmpbig{k}")
        if k == 1:
            # ACT affine for m1 runs while DVE finishes m0
            nc.scalar.activation(tmp[:], h2[:, 1, :], AF.Identity,
                                 bias=mb[:, 1:2], scale=scl[:, 1:2])
        else:
            nc.vector.tensor_scalar(
                out=tmp[:], in0=h2[:, k, :], scalar1=scl[:, k:k + 1],
                scalar2=mb[:, k:k + 1], op0=ALU.mult, op1=ALU.add)
        fin = wk1.tile([128, T], F32, tag=f"fin{k}")
        for th in range(2):
            sl = slice(512 * th, 512 * th + 512)
            nc.vector.tensor_tensor(
                out=fin[:, sl], in0=tmp[:, sl],
                in1=out1[:, k, sl].bitcast(F32), op=ALU.add)
            eng = nc.sync if th == 0 else nc.gpsimd
            eng.dma_start(out=outv[:, k, sl], in_=fin[:, sl])


# ---------------------------------------------------------------- build
def build_kernel(loop_R=None, fake_collectives=False):
    nc = bacc.Bacc("TRN2", target_bir_lowering=False, debug=False,
                   num_devices=N_CORES)
    xb_e = nc.dram_tensor("xb", [C, T], F32, kind="ExternalInput")
    whead_e = nc.dram_tensor("whead", [65, 4, 96], F32, kind="ExternalInput")
    wo_e = nc.dram_tensor("wo_p", [65, 4, 64], F32, kind="ExternalInput")
    obc_e = nc.dram_tensor("ob_col", [128, 4], F32, kind="ExternalInput")
    wf1_e = nc.dram_tensor("wf1", [C, 4 * C], F32, kind="ExternalInput")
    bf1_e = nc.dram_tensor("bf1", [128, 8], F32, kind="ExternalInput")
    wf2_e = nc.dram_tensor("wf2", [4 * C, C], F32, kind="ExternalInput")
    gb1_e = nc.dram_tensor("gb1", [128, 2, 2], F32, kind="ExternalInput")
    gb2_e = nc.dram_tensor("gb2", [128, 2, 2], F32, kind="ExternalInput")
    rsl_e = nc.dram_tensor("rslots", [1, 3], mybir.dt.uint32,
                           kind="ExternalInput")
    out_e = nc.dram_tensor("out", [C, T], F32, kind="ExternalOutput")

    import contextlib
    with tile.TileContext(nc) as tc, contextlib.ExitStack() as ctx:
        consts = ctx.enter_context(tc.tile_pool(name="consts", bufs=1))
        state = ctx.enter_context(tc.tile_pool(name="state", bufs=1))
        work = ctx.enter_context(tc.tile_pool(name="work", bufs=2))
        wk1 = ctx.enter_context(tc.tile_pool(name="wk1", bufs=1))
        expp = ctx.enter_context(tc.tile_pool(name="expp", bufs=3))
        psum = ctx.enter_context(tc.tile_pool(name="psum", bufs=1,
                                              space="PSUM"))
        dram = ctx.enter_context(tc.tile_pool(name="dram", bufs=2,
                                              space="DRAM"))

        def load_round(tag, shape, e_ap, rearr=None):
            stg = wk1.tile(shape, F32, tag="stage")
            srcap = e_ap if rearr is None else e_ap.rearrange(rearr, p=128)
            nc.sync.dma_start(out=stg[:], in_=srcap)
            r = consts.tile(shape, F32R, tag=tag)
            nc.vector.tensor_copy(r[:], stg[:])
            return r

        # whead first: the head-0 projections (and thus AG0) gate everything
        whead = load_round("whead", [65, 4, 96], whead_e.ap())
        x_sb = consts.tile([128, 2, T], F32)
        xview = xb_e.ap().rearrange("(k p) t -> p k t", p=128)
        nc.sync.dma_start(out=x_sb[0:64, 0, :], in_=xview[0:64, 0, :])
        nc.gpsimd.dma_start(out=x_sb[:, 1, :], in_=xview[:, 1, :])
        wo = load_round("wo", [65, 4, 64], wo_e.ap())
        obc = consts.tile([128, 4], F32)
        nc.sync.dma_start(out=obc[:], in_=obc_e[:, :])
        nc.sync.dma_start(out=x_sb[64:128, 0, :], in_=xview[64:128, 0, :])
        xob = consts.tile([128, 2, T], F32)
        # FFN weights: DMAs deferred (gpsimd queue, after AG0 launch) and the
        # f32r rounding copies off the startup-critical DVE queue (only
        # needed ~200us later).
        wf1_stg = wk1.tile([128, 2, 1024], F32, tag="stage1")
        wf1b = consts.tile([128, 2, 1024], BF16, tag="wf1b")
        wf2_stg = wk1.tile([128, 8, 256], F32, tag="stage2")
        wf2b = consts.tile([128, 8, 256], BF16, tag="wf2b")
        bf1 = consts.tile([128, 8], F32)
        gb1 = consts.tile([128, 2, 2], F32)
        gb2 = consts.tile([128, 2, 2], F32)

        def load_late_weights():
            nc.gpsimd.dma_start(out=bf1[:], in_=bf1_e[:, :])
            nc.gpsimd.dma_start(out=gb1[:], in_=gb1_e[:, :, :])
            nc.gpsimd.dma_start(out=gb2[:], in_=gb2_e[:, :, :])
            nc.gpsimd.dma_start(
                out=wf1_stg[:],
                in_=wf1_e.ap().rearrange("(k p) m -> p k m", p=128))
            nc.gpsimd.dma_start(
                out=wf2_stg[:],
                in_=wf2_e.ap().rearrange("(k p) m -> p k m", p=128))

        def finish_weights():
            nc.vector.tensor_copy(wf1b[:], wf1_stg[:])
            nc.vector.tensor_copy(wf2b[:], wf2_stg[:])
        magic = consts.tile([128, 2], I32)
        nc.vector.memset(magic[:], RSQRT_MAGIC)
        slots = []
        slots_p = []
        for k in range(3):
            tmp = nc.sync.alloc_register(f"rslot{k}")
            nc.sync.reg_load(tmp, rsl_e[0:1, k:k + 1])
            slots.append(nc.sync.snap(tmp, donate=True, min_val=0, max_val=3))
            tmpp = nc.gpsimd.alloc_register(f"rslotp{k}")
            nc.gpsimd.reg_load(tmpp, rsl_e[0:1, k:k + 1])
            slots_p.append(nc.gpsimd.snap(tmpp, donate=True, min_val=0,
                                          max_val=3))

        G = dict(x_sb=x_sb, whead=whead, wo=wo, wf1b=wf1b, bf1=bf1,
                 wf2b=wf2b, gb1=gb1, gb2=gb2, magic=magic, slots=slots,
                 slots_p=slots_p, obc=obc, xob=xob,
                 work=work, wk1=wk1, expp=expp, psum=psum, dram=dram,
                 state=state, fake=fake_collectives)

        def compute(it):
            cc_t = state.tile([128, 2, T], F32, tag="cc")
            stat_t = state.tile([128, 2, 2], F32, tag="stat")
            out1_t = state.tile([128, 2, T], F32R, tag="out1")
            out1b_t = state.tile([128, 2, T], BF16, tag="out1b")
            G["cc"], G["stat"], G["out1"] = cc_t, stat_t, out1_t
            G["out1b"] = out1b_t
            xiT0 = work.tile([65, T], F32R, tag="xiT")
            _memset_r(nc, xiT0[64:65, :], 1.0)
            qlkl0 = work.tile([16, 2, T], BF16, tag="qlkl")
            v_own0 = work.tile([128, 8, 65], BF16, tag="vown")
            ag0 = []
            for h in range(2):
                hs = slice(512 * h, 512 * h + 512)
                nc.vector.tensor_copy(xiT0[0:64, hs], x_sb[0:64, 0, hs])
                _proj_half(nc, G, 0, h, xiT0, qlkl0)
                _v_half(nc, G, 0, h, xiT0, v_own0)
                ag0.append(_launch_ag_half(nc, G, h, qlkl0, v_own0))
            (klfA0, vtA0), (klfB0, vtB0) = ag0
            if isinstance(it, int) and it == 0:
                load_late_weights()
                finish_weights()
            for i4 in range(4):
                hi4, lo4 = divmod(i4, 2)
                q0 = 64 * lo4
                nc.vector.tensor_scalar_add(
                    xob[q0:q0 + 64, hi4, :], x_sb[q0:q0 + 64, hi4, :],
                    obc[q0:q0 + 64, i4:i4 + 1])
            outa0 = psum.tile([65, 512], F32, tag="outA")
            outb0 = psum.tile([65, 512], F32, tag="outB")
            hstate = (xiT0, qlkl0, v_own0, klfA0, vtA0, klfB0, vtB0,
                      (outa0, outb0), [False, False], {0: 32, 1: 32},
                      0, 0, None)
            for i in range(4):
                hstate = _head(nc, G, i, hstate)
            _ffn_tail(nc, G, out_e)

        if loop_R is None:
            compute(0)
        else:
            load_late_weights()
            finish_weights()
            for i4 in range(4):
                hi4, lo4 = divmod(i4, 2)
                q0 = 64 * lo4
                nc.vector.tensor_scalar_add(
                    xob[q0:q0 + 64, hi4, :], x_sb[q0:q0 + 64, hi4, :],
                    obc[q0:q0 + 64, i4:i4 + 1])
            with tc.For_i(0, loop_R, 1,
                          hint_engines=(mybir.EngineType.PE,
                                        mybir.EngineType.Activation,
                                        mybir.EngineType.DVE,
                                        mybir.EngineType.SP,
                                        mybir.EngineType.Pool)) as it:
                compute(it)
    nc.compile()
    return nc


# ---------------------------------------------------------------- driver
_CACHED_NC = None


def _get_nc():
    global _CACHED_NC
    if _CACHED_NC is None:
        _CACHED_NC = build_kernel(loop_R=None, fake_collectives=False)
    return _CACHED_NC


def kernel(**inputs):
    """Full (unsharded) reference inputs -> full [2, 256, 64, 64] output.

    Shards batch x sequence across the 8 NeuronCores, runs the Bass kernel
    via run_bass_kernel_spmd, and reassembles the output."""
    from concourse.bass_utils import run_bass_kernel_spmd

    inputs = {k: np.asarray(v) for k, v in inputs.items()}
    in_maps = prep_host_inputs(**inputs)
    nc = _get_nc()
    res = run_bass_kernel_spmd(nc, in_maps, core_ids=list(range(N_CORES)))
    return assemble_output(res.results)

